# revision 5
# baseline (speedup 1.0000x reference)
"""Gemma4 attention layer on 8 TRN2 NeuronCores, tensor-parallel over heads.

Per core c: q-heads {2c, 2c+1}, kv-head c//2. All matmuls in float32r
(~tf32 precision, 1 cyc/row at N>=512). Host shards/transposes inputs,
device computes yT partial = (attn @ o_w_shard).T, host sums partials.
"""

import sys

sys.path.insert(0, "/opt/trn_rl_repo")

from contextlib import ExitStack

import numpy as np

import concourse.bass as bass
import concourse.tile as tile
from concourse import mybir, bacc
from concourse.bass_utils import run_bass_kernel_spmd
from concourse.masks import make_identity

F32 = mybir.dt.float32
F32R = mybir.dt.float32r

B, T, HID = 1, 1024, 2048
NH, NKV, HD = 16, 4, 512
ROT = 128
THETA = 1000000.0
EPS = 1e-6
NEG = -1e30
NC_ = 8           # cores
HPC = NH // NC_   # q heads per core = 2
DQ = HPC * HD     # 1024 per-core q width
TT = T // 128     # 8 t-tiles
HCH = HID // 128  # 16 hidden chunks


def build_kernel():
    nc = bacc.Bacc("TRN2", target_bir_lowering=False, debug=False, num_devices=NC_)
    xT = nc.dram_tensor("xT", [HID, T], F32, kind="ExternalInput")
    qwT = nc.dram_tensor("qwT", [HID, DQ], F32, kind="ExternalInput")
    kwT = nc.dram_tensor("kwT", [HID, HD], F32, kind="ExternalInput")
    owT = nc.dram_tensor("owT", [DQ, HID], F32, kind="ExternalInput")
    cosw = nc.dram_tensor("cosw", [T, ROT], F32, kind="ExternalInput")
    sinw = nc.dram_tensor("sinw", [T, ROT], F32, kind="ExternalInput")  # sign-baked
    m4 = nc.dram_tensor("m4", [4, 128, 512], F32, kind="ExternalInput")
    qnw = nc.dram_tensor("qnw", [512], F32, kind="ExternalInput")
    knw = nc.dram_tensor("knw", [512], F32, kind="ExternalInput")
    yT = nc.dram_tensor("yT", [HID, T], F32, kind="ExternalOutput")

    with tile.TileContext(nc) as tc:
        _body(nc, tc, xT, qwT, kwT, owT, cosw, sinw, m4, qnw, knw, yT)
    nc.compile()
    return nc


def _bcast_ap(dram_ap, parts):
    return bass.AP(
        tensor=dram_ap.tensor,
        offset=dram_ap.offset,
        ap=[[0, parts]] + list(dram_ap.ap),
    )


def _body(nc, tc, xT, qwT, kwT, owT, cosw, sinw, m4, qnw, knw, yT):
    cp = [0]  # copy-engine round robin

    def pcopy(dst, src, small=False):
        if small or cp[0] % 2 == 0:
            nc.vector.tensor_copy(dst, src)
        else:
            nc.scalar.copy(dst, src)
        if not small:
            cp[0] += 1

    with ExitStack() as root:
        const = root.enter_context(tc.tile_pool(name="const", bufs=1))
        ident = const.tile([128, 128], F32)
        make_identity(nc, ident[:])
        qnw_b = const.tile([128, 512], F32)
        nc.sync.dma_start(out=qnw_b[:], in_=_bcast_ap(qnw.ap(), 128))
        knw_b = const.tile([128, 512], F32)
        nc.sync.dma_start(out=knw_b[:], in_=_bcast_ap(knw.ap(), 128))
        eps_t = const.tile([128, 1], F32)
        nc.vector.memset(eps_t[:], EPS)
        zeros_t = const.tile([128, T], F32)
        nc.vector.memset(zeros_t[:], 0.0)
        cos_all = const.tile([128, TT, ROT], F32)
        nc.sync.dma_start(out=cos_all[:], in_=cosw.ap().rearrange("(n p) d -> p n d", p=128))
        sin_all = const.tile([128, TT, ROT], F32)
        nc.sync.dma_start(out=sin_all[:], in_=sinw.ap().rearrange("(n p) d -> p n d", p=128))
        m4_sb = const.tile([128, 4, 512], F32)
        nc.sync.dma_start(out=m4_sb[:], in_=m4.ap().rearrange("m p s -> p m s"))

        # ============ phase A: projections, then norm/rope/transpose =======
        qtkv = root.enter_context(tc.tile_pool(name="qtkv", bufs=1))
        qT_r = qtkv.tile([128, 2 * 4, T], F32R)     # 32KB
        kT_r = qtkv.tile([128, 4, T], F32R)         # 16KB
        v_r = qtkv.tile([128, TT, HD], F32R)        # 16KB

        with ExitStack() as pa:
            projdata = pa.enter_context(tc.tile_pool(name="projdata", bufs=1))
            q_all = projdata.tile([128, TT, DQ], F32)   # 32KB/part
            k_all = projdata.tile([128, TT, HD], F32)   # 16KB/part

            with ExitStack() as pa1:
                xpool = pa1.enter_context(tc.tile_pool(name="xTp", bufs=1))
                xT_r = xpool.tile([128, HCH, T], F32R)  # 64KB/part
                for h in range(0, HCH, 4):
                    nc.gpsimd.dma_start(
                        out=xT_r[:, h : h + 4, :],
                        in_=xT.ap().rearrange("(n p) t -> p n t", p=128)[:, h : h + 4, :],
                    )
                wpool = pa1.enter_context(tc.tile_pool(name="w", bufs=3))
                pps = pa1.enter_context(tc.tile_pool(name="proj_ps", bufs=1, space="PSUM"))
                psq = []
                for i in range(TT):
                    pst = pps.tile([128, 512], F32, tag=f"ps{i}", name=f"psq{i}")
                    psq.append(pst)

                def proj_pass(w_dram_slice, dst_view):
                    for h in range(HCH):
                        wt = wpool.tile([128, 512], F32R, tag="w")
                        nc.gpsimd.dma_start(out=wt[:], in_=w_dram_slice(h))
                        for i in range(TT):
                            nc.tensor.matmul(
                                psq[i][:],
                                xT_r[:, h, i * 128 : (i + 1) * 128],
                                wt[:],
                                start=(h == 0),
                                stop=(h == HCH - 1),
                            )
                    for i in range(TT):
                        pcopy(dst_view(i), psq[i][:])

                for dqh in range(2):
                    proj_pass(
                        lambda h, dqh=dqh: qwT.ap()[
                            h * 128 : (h + 1) * 128, dqh * 512 : (dqh + 1) * 512
                        ],
                        lambda i, dqh=dqh: q_all[:, i, dqh * 512 : (dqh + 1) * 512],
                    )
                proj_pass(
                    lambda h: kwT.ap()[h * 128 : (h + 1) * 128, :],
                    lambda i: k_all[:, i, :],
                )

            with ExitStack() as pa2:
                tmp = pa2.enter_context(tc.tile_pool(name="tmp", bufs=3))
                tp_ps = pa2.enter_context(tc.tile_pool(name="tp_ps", bufs=4, space="PSUM"))

                def norm_rope_transpose(blk, w_b, i, dst, dst_idx0, is_v_source):
                    sq = tmp.tile([128, 512], F32, tag="sq")
                    nc.vector.tensor_mul(sq[:], blk, blk)
                    st = tmp.tile([128, 6], F32, tag="st")
                    nc.vector.bn_stats(out=st[:], in_=sq[:])
                    mv = tmp.tile([128, 2], F32, tag="mv")
                    nc.vector.bn_aggr(out=mv[:], in_=st[:])
                    sd = tmp.tile([128, 1], F32, tag="sd")
                    nc.scalar.activation(
                        out=sd[:], in_=mv[:, 0:1],
                        func=mybir.ActivationFunctionType.Sqrt,
                        bias=eps_t[:], scale=1.0,
                    )
                    rs = tmp.tile([128, 1], F32, tag="rs")
                    nc.vector.reciprocal(out=rs[:], in_=sd[:])
                    if is_v_source:
                        nc.vector.tensor_scalar_mul(out=v_r[:, i, :], in0=blk, scalar1=rs[:])
                    xn = tmp.tile([128, 512], F32, tag="xn")
                    nc.vector.tensor_scalar_mul(out=xn[:], in0=blk, scalar1=rs[:])
                    nc.vector.tensor_mul(xn[:], xn[:], w_b[:])
                    c = cos_all[:, i, :]
                    s = sin_all[:, i, :]
                    t1 = tmp.tile([128, 128], F32, tag="t1")
                    nc.vector.tensor_mul(t1[:], xn[:, 0:ROT], c)
                    t2 = tmp.tile([128, 128], F32, tag="t2")
                    nc.vector.tensor_mul(t2[:, 0:64], xn[:, 64:128], s[:, 0:64])
                    nc.vector.tensor_mul(t2[:, 64:128], xn[:, 0:64], s[:, 64:128])
                    rot = tmp.tile([128, 128], F32, tag="rot")
                    nc.vector.tensor_add(rot[:], t1[:], t2[:])
                    for d4 in range(4):
                        src = rot[:] if d4 == 0 else xn[:, d4 * 128 : (d4 + 1) * 128]
                        tp = tp_ps.tile([128, 128], F32, tag="tp")
                        nc.tensor.transpose(tp[:], src, ident[:])
                        pcopy(dst[:, dst_idx0 + d4, i * 128 : (i + 1) * 128], tp[:], small=True)

                for i in range(TT):
                    norm_rope_transpose(k_all[:, i, :], knw_b, i, kT_r, 0, True)
                    for hh in range(HPC):
                        norm_rope_transpose(
                            q_all[:, i, hh * 512 : (hh + 1) * 512], qnw_b, i, qT_r, hh * 4, False
                        )

        # ============ phase B: attention per head ==========================
        outp = root.enter_context(tc.tile_pool(name="outp", bufs=1))
        outT_r = outp.tile([128, 2 * 4, T], F32R)       # 32KB

        with ExitStack() as pb:
            pTpool = pb.enter_context(tc.tile_pool(name="pTp", bufs=1))
            pT_r = pTpool.tile([128, TT, T], F32R)      # 32KB
            sc_ps = pb.enter_context(tc.tile_pool(name="sc_ps", bufs=3, space="PSUM"))
            tp2_ps = pb.enter_context(tc.tile_pool(name="tp2_ps", bufs=3, space="PSUM"))
            pv_ps = pb.enter_context(tc.tile_pool(name="pv_ps", bufs=2, space="PSUM"))
            sm = pb.enter_context(tc.tile_pool(name="sm", bufs=2))

            for hh in range(HPC):
                for j in range(1, TT):
                    nc.vector.tensor_copy(pT_r[:, j, 0 : 128 * j], zeros_t[:, 0 : 128 * j])
                for i in range(TT):
                    nsh = i // 4 + 1
                    pss = []
                    mj = sm.tile([128, 2], F32, tag="mj")
                    for sh in range(nsh):
                        ps = sc_ps.tile([128, 512], F32, tag="pss")
                        for d4 in range(4):
                            nc.tensor.matmul(
                                ps[:],
                                qT_r[:, hh * 4 + d4, i * 128 : (i + 1) * 128],
                                kT_r[:, d4, sh * 512 : (sh + 1) * 512],
                                start=(d4 == 0),
                                stop=(d4 == 3),
                            )
                        if sh == i // 4:
                            nc.vector.tensor_add(ps[:], ps[:], m4_sb[:, i % 4, :])
                        nc.vector.tensor_reduce(
                            out=mj[:, sh : sh + 1], in_=ps[:],
                            op=mybir.AluOpType.max, axis=mybir.AxisListType.X,
                        )
                        pss.append(ps)
                    negm = sm.tile([128, 1], F32, tag="negm")
                    if nsh == 2:
                        m_c = sm.tile([128, 1], F32, tag="mc")
                        nc.vector.tensor_tensor(
                            out=m_c[:], in0=mj[:, 0:1], in1=mj[:, 1:2],
                            op=mybir.AluOpType.max,
                        )
                        nc.scalar.mul(negm[:], m_c[:], -1.0)
                    else:
                        nc.scalar.mul(negm[:], mj[:, 0:1], -1.0)
                    lp = sm.tile([128, 2], F32, tag="lp")
                    es = []
                    for sh in range(nsh):
                        e_sb = sm.tile([128, 512], F32, tag=f"e{sh}")
                        nc.scalar.activation(
                            out=e_sb[:], in_=pss[sh][:],
                            func=mybir.ActivationFunctionType.Exp,
                            bias=negm[:], scale=1.0,
                            accum_out=lp[:, sh : sh + 1],
                        )
                        es.append(e_sb)
                    lsum = sm.tile([128, 1], F32, tag="lsum")
                    if nsh == 2:
                        nc.vector.tensor_add(lsum[:], lp[:, 0:1], lp[:, 1:2])
                    else:
                        nc.vector.tensor_copy(lsum[:], lp[:, 0:1])
                    rinv = sm.tile([128, 1], F32, tag="rinv")
                    nc.vector.reciprocal(out=rinv[:], in_=lsum[:])
                    for sh in range(nsh):
                        nc.vector.tensor_scalar_mul(out=es[sh][:], in0=es[sh][:], scalar1=rinv[:])
                        for b in range(4):
                            j = sh * 4 + b
                            if j > i:
                                break
                            tp = tp2_ps.tile([128, 128], F32, tag="tp2")
                            nc.tensor.transpose(tp[:], es[sh][:, b * 128 : (b + 1) * 128], ident[:])
                            pcopy(pT_r[:, j, i * 128 : (i + 1) * 128], tp[:], small=True)
                for th in range(2):
                    jmax = 4 if th == 0 else 8
                    for d4 in range(4):
                        ps = pv_ps.tile([128, 512], F32, tag="pso")
                        for j in range(jmax):
                            nc.tensor.matmul(
                                ps[:],
                                v_r[:, j, d4 * 128 : (d4 + 1) * 128],
                                pT_r[:, j, th * 512 : (th + 1) * 512],
                                start=(j == 0),
                                stop=(j == jmax - 1),
                            )
                        pcopy(outT_r[:, hh * 4 + d4, th * 512 : (th + 1) * 512], ps[:])

        # ============ phase C: o_proj ======================================
        with ExitStack() as pc:
            owpool = pc.enter_context(tc.tile_pool(name="ow", bufs=8))
            y_ps = pc.enter_context(tc.tile_pool(name="y_ps", bufs=3, space="PSUM"))
            ypool = pc.enter_context(tc.tile_pool(name="yst", bufs=4))
            ow_t = []
            for dc in range(8):
                wt = owpool.tile([128, HID], F32R, tag="ow")
                nc.gpsimd.dma_start(out=wt[:], in_=owT.ap()[dc * 128 : (dc + 1) * 128, :])
                ow_t.append(wt)
            for ec in range(HID // 128):
                for th in range(2):
                    ps = y_ps.tile([128, 512], F32, tag="psy")
                    for dc in range(8):
                        nc.tensor.matmul(
                            ps[:],
                            ow_t[dc][:, ec * 128 : (ec + 1) * 128],
                            outT_r[:, dc, th * 512 : (th + 1) * 512],
                            start=(dc == 0),
                            stop=(dc == 7),
                        )
                    yst = ypool.tile([128, 512], F32, tag="yst")
                    pcopy(yst[:], ps[:])
                    nc.sync.dma_start(
                        out=yT.ap()[ec * 128 : (ec + 1) * 128, th * 512 : (th + 1) * 512],
                        in_=yst[:],
                    )


_NC_CACHE = None


def _get_nc():
    global _NC_CACHE
    if _NC_CACHE is None:
        _NC_CACHE = build_kernel()
    return _NC_CACHE


def make_in_maps(x, q_w, k_w, o_w, q_norm_w, k_norm_w, input_pos):
    x = np.asarray(x)
    q_w = np.asarray(q_w)
    k_w = np.asarray(k_w)
    o_w = np.asarray(o_w)
    q_norm_w = np.asarray(q_norm_w, dtype=np.float32)
    k_norm_w = np.asarray(k_norm_w, dtype=np.float32)
    pos = np.asarray(input_pos)

    x2 = np.ascontiguousarray(x.reshape(T, HID).astype(np.float32))
    xT = np.ascontiguousarray(x2.T)

    posf = pos.astype(np.float32)
    inv_freq = (1.0 / (THETA ** (np.arange(0, ROT, 2, dtype=np.float32) / ROT))).astype(np.float32)
    freqs = posf[:, None] * inv_freq[None, :]
    emb = np.concatenate([freqs, freqs], axis=-1)
    cosw = np.cos(emb).astype(np.float32)
    sinw = np.sin(emb).astype(np.float32)
    sin_signed = sinw.copy()
    sin_signed[:, : ROT // 2] = -sin_signed[:, : ROT // 2]

    r_ = np.arange(4)[:, None, None]
    p_ = np.arange(128)[None, :, None]
    f_ = np.arange(512)[None, None, :]
    m4 = np.where(f_ <= 128 * r_ + p_, 0.0, NEG).astype(np.float32)

    in_maps = []
    for c in range(NC_):
        g = c // 2
        qwT = np.ascontiguousarray(
            q_w[2 * c * HD : (2 * c + 2) * HD, :].astype(np.float32).T
        )
        kwT = np.ascontiguousarray(k_w[g * HD : (g + 1) * HD, :].astype(np.float32).T)
        owT = np.ascontiguousarray(
            o_w[:, 2 * c * HD : (2 * c + 2) * HD].astype(np.float32).T
        )
        in_maps.append(
            {
                "xT": xT, "qwT": qwT, "kwT": kwT, "owT": owT,
                "cosw": cosw, "sinw": sin_signed, "m4": m4,
                "qnw": q_norm_w, "knw": k_norm_w,
            }
        )
    return in_maps


def kernel(x, q_w, k_w, o_w, q_norm_w, k_norm_w, input_pos):
    pos = np.asarray(input_pos)
    assert np.array_equal(pos, np.arange(T)), "kernel assumes input_pos == arange(T)"
    nc = _get_nc()
    in_maps = make_in_maps(x, q_w, k_w, o_w, q_norm_w, k_norm_w, input_pos)
    res = run_bass_kernel_spmd(nc, in_maps, list(range(NC_)))
    acc = np.zeros((T, HID), dtype=np.float64)
    for c in range(NC_):
        acc += res.results[c]["yT"].T
    return acc.astype(np.float32).reshape(B, T, HID)


# revision 14
# speedup vs baseline: 68.2178x; 68.2178x over previous
"""Gemma4 attention layer on 8 TRN2 NeuronCores, tensor-parallel over heads.

Per core c: q-heads {2c, 2c+1}, kv-head c//2. All matmuls in float32r
(~tf32 precision, 1 cyc/row at N>=512). Host shards/transposes inputs,
device computes yT partial = (attn @ o_w_shard).T, host sums partials.
"""

import sys

sys.path.insert(0, "/opt/trn_rl_repo")

from contextlib import ExitStack

import numpy as np

import concourse.bass as bass
import concourse.tile as tile
from concourse import mybir, bacc
from concourse.bass_utils import run_bass_kernel_spmd
from concourse.masks import make_identity

F32 = mybir.dt.float32
F32R = mybir.dt.float32r

B, T, HID = 1, 1024, 2048
NH, NKV, HD = 16, 4, 512
ROT = 128
THETA = 1000000.0
EPS = 1e-6
NEG = -1e30
NC_ = 8           # cores
HPC = NH // NC_   # q heads per core = 2
DQ = HPC * HD     # 1024 per-core q width
TT = T // 128     # 8 t-tiles
HCH = HID // 128  # 16 hidden chunks


def build_kernel(n_rep=1):
    nc = bacc.Bacc("TRN2", target_bir_lowering=False, debug=False, num_devices=NC_)
    xT = nc.dram_tensor("xT", [HID, T], F32, kind="ExternalInput")
    qwT = nc.dram_tensor("qwT", [HID, DQ], F32, kind="ExternalInput")
    kwT = nc.dram_tensor("kwT", [HID, HD], F32, kind="ExternalInput")
    owT = nc.dram_tensor("owT", [DQ, HID], F32, kind="ExternalInput")
    cosw = nc.dram_tensor("cosw", [T, ROT], F32, kind="ExternalInput")
    sinw = nc.dram_tensor("sinw", [T, ROT], F32, kind="ExternalInput")  # sign-baked
    m4 = nc.dram_tensor("m4", [4, 128, 512], F32, kind="ExternalInput")
    qnw = nc.dram_tensor("qnw", [512], F32, kind="ExternalInput")
    knw = nc.dram_tensor("knw", [512], F32, kind="ExternalInput")
    yT = nc.dram_tensor("yT", [HID, T], F32, kind="ExternalOutput")

    with tile.TileContext(nc) as tc:
        for _rep in range(n_rep):
            _body(nc, tc, xT, qwT, kwT, owT, cosw, sinw, m4, qnw, knw, yT)
    nc.compile()
    return nc


def _bcast_ap(dram_ap, parts):
    return bass.AP(
        tensor=dram_ap.tensor,
        offset=dram_ap.offset,
        ap=[[0, parts]] + list(dram_ap.ap),
    )


def _body(nc, tc, xT, qwT, kwT, owT, cosw, sinw, m4, qnw, knw, yT):
    cp = [0]  # copy-engine round robin

    def pcopy(dst, src, small=False):
        if small or cp[0] % 2 == 0:
            nc.vector.tensor_copy(dst, src)
        else:
            nc.scalar.copy(dst, src)
        if not small:
            cp[0] += 1

    with ExitStack() as root:
        const = root.enter_context(tc.tile_pool(name="const", bufs=1))
        ident = const.tile([128, 128], F32)
        make_identity(nc, ident[:])
        qnw_b = const.tile([128, 512], F32)
        nc.sync.dma_start(out=qnw_b[:], in_=_bcast_ap(qnw.ap(), 128))
        knw_b = const.tile([128, 512], F32)
        nc.sync.dma_start(out=knw_b[:], in_=_bcast_ap(knw.ap(), 128))
        eps_t = const.tile([128, 1], F32)
        nc.vector.memset(eps_t[:], EPS)
        zeros_t = const.tile([128, T], F32)
        nc.vector.memset(zeros_t[:], 0.0)
        cos_all = const.tile([128, TT, ROT], F32)
        nc.sync.dma_start(out=cos_all[:], in_=cosw.ap().rearrange("(n p) d -> p n d", p=128))
        sin_all = const.tile([128, TT, ROT], F32)
        nc.sync.dma_start(out=sin_all[:], in_=sinw.ap().rearrange("(n p) d -> p n d", p=128))
        m4_sb = const.tile([128, 4, 512], F32)
        nc.sync.dma_start(out=m4_sb[:], in_=m4.ap().rearrange("m p s -> p m s"))

        # ============ phase A: projections, then norm/rope/transpose =======
        qtkv = root.enter_context(tc.tile_pool(name="qtkv", bufs=1))
        qT_r = qtkv.tile([128, 2 * 4, T], F32R)     # 32KB
        kT_r = qtkv.tile([128, 4, T], F32R)         # 16KB
        v_r = qtkv.tile([128, TT, HD], F32R)        # 16KB

        with ExitStack() as pa:
            projdata = pa.enter_context(tc.tile_pool(name="projdata", bufs=1))
            q_all = projdata.tile([128, TT, DQ], F32)   # 32KB/part
            k_all = projdata.tile([128, TT, HD], F32)   # 16KB/part

            with ExitStack() as pa1:
                xpool = pa1.enter_context(tc.tile_pool(name="xTp", bufs=1))
                xT_r = xpool.tile([128, HCH, T], F32R)  # 64KB/part
                for h in range(0, HCH, 4):
                    nc.gpsimd.dma_start(
                        out=xT_r[:, h : h + 4, :],
                        in_=xT.ap().rearrange("(n p) t -> p n t", p=128)[:, h : h + 4, :],
                    )
                wpool = pa1.enter_context(tc.tile_pool(name="w", bufs=3))
                pps = pa1.enter_context(tc.tile_pool(name="proj_ps", bufs=1, space="PSUM"))
                psq = []
                for i in range(TT):
                    pst = pps.tile([128, 512], F32, tag=f"ps{i}", name=f"psq{i}")
                    psq.append(pst)

                def proj_pass(w_dram_slice, dst_view):
                    for h in range(HCH):
                        wt = wpool.tile([128, 512], F32R, tag="w")
                        nc.gpsimd.dma_start(out=wt[:], in_=w_dram_slice(h))
                        for i in range(TT):
                            nc.tensor.matmul(
                                psq[i][:],
                                xT_r[:, h, i * 128 : (i + 1) * 128],
                                wt[:],
                                start=(h == 0),
                                stop=(h == HCH - 1),
                            )
                    for i in range(TT):
                        pcopy(dst_view(i), psq[i][:])

                for dqh in range(2):
                    proj_pass(
                        lambda h, dqh=dqh: qwT.ap()[
                            h * 128 : (h + 1) * 128, dqh * 512 : (dqh + 1) * 512
                        ],
                        lambda i, dqh=dqh: q_all[:, i, dqh * 512 : (dqh + 1) * 512],
                    )
                proj_pass(
                    lambda h: kwT.ap()[h * 128 : (h + 1) * 128, :],
                    lambda i: k_all[:, i, :],
                )

            with ExitStack() as pa2:
                tmp = pa2.enter_context(tc.tile_pool(name="tmp", bufs=3))
                tp_ps = pa2.enter_context(tc.tile_pool(name="tp_ps", bufs=4, space="PSUM"))

                def norm_rope_transpose(blk, w_b, i, dst, dst_idx0, is_v_source):
                    sq = tmp.tile([128, 512], F32, tag="sq")
                    nc.vector.tensor_mul(sq[:], blk, blk)
                    st = tmp.tile([128, 6], F32, tag="st")
                    nc.vector.bn_stats(out=st[:], in_=sq[:])
                    mv = tmp.tile([128, 2], F32, tag="mv")
                    nc.vector.bn_aggr(out=mv[:], in_=st[:])
                    sd = tmp.tile([128, 1], F32, tag="sd")
                    nc.scalar.activation(
                        out=sd[:], in_=mv[:, 0:1],
                        func=mybir.ActivationFunctionType.Sqrt,
                        bias=eps_t[:], scale=1.0,
                    )
                    rs = tmp.tile([128, 1], F32, tag="rs")
                    nc.vector.reciprocal(out=rs[:], in_=sd[:])
                    if is_v_source:
                        nc.vector.tensor_scalar_mul(out=v_r[:, i, :], in0=blk, scalar1=rs[:])
                    xn = tmp.tile([128, 512], F32, tag="xn")
                    nc.vector.tensor_scalar_mul(out=xn[:], in0=blk, scalar1=rs[:])
                    nc.vector.tensor_mul(xn[:], xn[:], w_b[:])
                    c = cos_all[:, i, :]
                    s = sin_all[:, i, :]
                    t1 = tmp.tile([128, 128], F32, tag="t1")
                    nc.vector.tensor_mul(t1[:], xn[:, 0:ROT], c)
                    t2 = tmp.tile([128, 128], F32, tag="t2")
                    nc.vector.tensor_mul(t2[:, 0:64], xn[:, 64:128], s[:, 0:64])
                    nc.vector.tensor_mul(t2[:, 64:128], xn[:, 0:64], s[:, 64:128])
                    rot = tmp.tile([128, 128], F32, tag="rot")
                    nc.vector.tensor_add(rot[:], t1[:], t2[:])
                    for d4 in range(4):
                        src = rot[:] if d4 == 0 else xn[:, d4 * 128 : (d4 + 1) * 128]
                        tp = tp_ps.tile([128, 128], F32, tag="tp")
                        nc.tensor.transpose(tp[:], src, ident[:])
                        pcopy(dst[:, dst_idx0 + d4, i * 128 : (i + 1) * 128], tp[:], small=True)

                for i in range(TT):
                    norm_rope_transpose(k_all[:, i, :], knw_b, i, kT_r, 0, True)
                    for hh in range(HPC):
                        norm_rope_transpose(
                            q_all[:, i, hh * 512 : (hh + 1) * 512], qnw_b, i, qT_r, hh * 4, False
                        )

        # ============ phase B: attention per head ==========================
        outp = root.enter_context(tc.tile_pool(name="outp", bufs=1))
        outT_r = outp.tile([128, 2 * 4, T], F32R)       # 32KB

        with ExitStack() as pb:
            pTpool = pb.enter_context(tc.tile_pool(name="pTp", bufs=1))
            pT_heads = []
            for hh in range(HPC):
                pT_h = pTpool.tile([128, TT, T], F32R, tag=f"pT{hh}", name=f"pT{hh}")
                pT_heads.append(pT_h)
            sc_ps = pb.enter_context(tc.tile_pool(name="sc_ps", bufs=3, space="PSUM"))
            tp2_ps = pb.enter_context(tc.tile_pool(name="tp2_ps", bufs=3, space="PSUM"))
            pv_ps = pb.enter_context(tc.tile_pool(name="pv_ps", bufs=2, space="PSUM"))
            sm = pb.enter_context(tc.tile_pool(name="sm", bufs=2))

            for hh in range(HPC):
                pT_r = pT_heads[hh]
                for j in range(1, TT):
                    nc.vector.tensor_copy(pT_r[:, j, 0 : 128 * j], zeros_t[:, 0 : 128 * j])
                for i in range(TT):
                    nsh = i // 4 + 1
                    pss = []
                    mj = sm.tile([128, 2], F32, tag="mj")
                    for sh in range(nsh):
                        ps = sc_ps.tile([128, 512], F32, tag="pss")
                        for d4 in range(4):
                            nc.tensor.matmul(
                                ps[:],
                                qT_r[:, hh * 4 + d4, i * 128 : (i + 1) * 128],
                                kT_r[:, d4, sh * 512 : (sh + 1) * 512],
                                start=(d4 == 0),
                                stop=(d4 == 3),
                            )
                        if sh == i // 4:
                            nc.vector.tensor_add(ps[:], ps[:], m4_sb[:, i % 4, :])
                        nc.vector.tensor_reduce(
                            out=mj[:, sh : sh + 1], in_=ps[:],
                            op=mybir.AluOpType.max, axis=mybir.AxisListType.X,
                        )
                        pss.append(ps)
                    negm = sm.tile([128, 1], F32, tag="negm")
                    if nsh == 2:
                        m_c = sm.tile([128, 1], F32, tag="mc")
                        nc.vector.tensor_tensor(
                            out=m_c[:], in0=mj[:, 0:1], in1=mj[:, 1:2],
                            op=mybir.AluOpType.max,
                        )
                        nc.scalar.mul(negm[:], m_c[:], -1.0)
                    else:
                        nc.scalar.mul(negm[:], mj[:, 0:1], -1.0)
                    lp = sm.tile([128, 2], F32, tag="lp")
                    es = []
                    for sh in range(nsh):
                        e_sb = sm.tile([128, 512], F32, tag=f"e{sh}")
                        nc.scalar.activation(
                            out=e_sb[:], in_=pss[sh][:],
                            func=mybir.ActivationFunctionType.Exp,
                            bias=negm[:], scale=1.0,
                            accum_out=lp[:, sh : sh + 1],
                        )
                        es.append(e_sb)
                    lsum = sm.tile([128, 1], F32, tag="lsum")
                    if nsh == 2:
                        nc.vector.tensor_add(lsum[:], lp[:, 0:1], lp[:, 1:2])
                    else:
                        nc.vector.tensor_copy(lsum[:], lp[:, 0:1])
                    rinv = sm.tile([128, 1], F32, tag="rinv")
                    nc.vector.reciprocal(out=rinv[:], in_=lsum[:])
                    for sh in range(nsh):
                        nc.vector.tensor_scalar_mul(out=es[sh][:], in0=es[sh][:], scalar1=rinv[:])
                        for b in range(4):
                            j = sh * 4 + b
                            if j > i:
                                break
                            tp = tp2_ps.tile([128, 128], F32, tag="tp2")
                            nc.tensor.transpose(tp[:], es[sh][:, b * 128 : (b + 1) * 128], ident[:])
                            pcopy(pT_r[:, j, i * 128 : (i + 1) * 128], tp[:], small=True)
                for th in range(2):
                    jmax = 4 if th == 0 else 8
                    for d4 in range(4):
                        ps = pv_ps.tile([128, 512], F32, tag="pso")
                        for j in range(jmax):
                            nc.tensor.matmul(
                                ps[:],
                                v_r[:, j, d4 * 128 : (d4 + 1) * 128],
                                pT_r[:, j, th * 512 : (th + 1) * 512],
                                start=(j == 0),
                                stop=(j == jmax - 1),
                            )
                        pcopy(outT_r[:, hh * 4 + d4, th * 512 : (th + 1) * 512], ps[:])

        # ============ phase C: o_proj ======================================
        with ExitStack() as pc:
            owpool = pc.enter_context(tc.tile_pool(name="ow", bufs=8))
            y_ps = pc.enter_context(tc.tile_pool(name="y_ps", bufs=3, space="PSUM"))
            ypool = pc.enter_context(tc.tile_pool(name="yst", bufs=4))
            ow_t = []
            for dc in range(8):
                wt = owpool.tile([128, HID], F32R, tag="ow")
                nc.gpsimd.dma_start(out=wt[:], in_=owT.ap()[dc * 128 : (dc + 1) * 128, :])
                ow_t.append(wt)
            for ec in range(HID // 128):
                for th in range(2):
                    ps = y_ps.tile([128, 512], F32, tag="psy")
                    for dc in range(8):
                        nc.tensor.matmul(
                            ps[:],
                            ow_t[dc][:, ec * 128 : (ec + 1) * 128],
                            outT_r[:, dc, th * 512 : (th + 1) * 512],
                            start=(dc == 0),
                            stop=(dc == 7),
                        )
                    yst = ypool.tile([128, 512], F32, tag="yst")
                    pcopy(yst[:], ps[:])
                    nc.sync.dma_start(
                        out=yT.ap()[ec * 128 : (ec + 1) * 128, th * 512 : (th + 1) * 512],
                        in_=yst[:],
                    )


_NC_CACHE = None


def _get_nc():
    global _NC_CACHE
    if _NC_CACHE is None:
        _NC_CACHE = build_kernel()
    return _NC_CACHE


def make_in_maps(x, q_w, k_w, o_w, q_norm_w, k_norm_w, input_pos):
    x = np.asarray(x)
    q_w = np.asarray(q_w)
    k_w = np.asarray(k_w)
    o_w = np.asarray(o_w)
    q_norm_w = np.asarray(q_norm_w, dtype=np.float32)
    k_norm_w = np.asarray(k_norm_w, dtype=np.float32)
    pos = np.asarray(input_pos)

    x2 = np.ascontiguousarray(x.reshape(T, HID).astype(np.float32))
    xT = np.ascontiguousarray(x2.T)

    posf = pos.astype(np.float32)
    inv_freq = (1.0 / (THETA ** (np.arange(0, ROT, 2, dtype=np.float32) / ROT))).astype(np.float32)
    freqs = posf[:, None] * inv_freq[None, :]
    emb = np.concatenate([freqs, freqs], axis=-1)
    cosw = np.cos(emb).astype(np.float32)
    sinw = np.sin(emb).astype(np.float32)
    sin_signed = sinw.copy()
    sin_signed[:, : ROT // 2] = -sin_signed[:, : ROT // 2]

    r_ = np.arange(4)[:, None, None]
    p_ = np.arange(128)[None, :, None]
    f_ = np.arange(512)[None, None, :]
    m4 = np.where(f_ <= 128 * r_ + p_, 0.0, NEG).astype(np.float32)

    in_maps = []
    for c in range(NC_):
        g = c // 2
        qwT = np.ascontiguousarray(
            q_w[2 * c * HD : (2 * c + 2) * HD, :].astype(np.float32).T
        )
        kwT = np.ascontiguousarray(k_w[g * HD : (g + 1) * HD, :].astype(np.float32).T)
        owT = np.ascontiguousarray(
            o_w[:, 2 * c * HD : (2 * c + 2) * HD].astype(np.float32).T
        )
        in_maps.append(
            {
                "xT": xT, "qwT": qwT, "kwT": kwT, "owT": owT,
                "cosw": cosw, "sinw": sin_signed, "m4": m4,
                "qnw": q_norm_w, "knw": k_norm_w,
            }
        )
    return in_maps


def kernel(x, q_w, k_w, o_w, q_norm_w, k_norm_w, input_pos):
    pos = np.asarray(input_pos)
    assert np.array_equal(pos, np.arange(T)), "kernel assumes input_pos == arange(T)"
    nc = _get_nc()
    in_maps = make_in_maps(x, q_w, k_w, o_w, q_norm_w, k_norm_w, input_pos)
    res = run_bass_kernel_spmd(nc, in_maps, list(range(NC_)))
    acc = np.zeros((T, HID), dtype=np.float64)
    for c in range(NC_):
        acc += res.results[c]["yT"].T
    return acc.astype(np.float32).reshape(B, T, HID)
